# revision 76
# baseline (speedup 1.0000x reference)
"""Trainium2 Bass kernel for nn_Mixture_24541443129646.

loss(x, mu, prec) = -sum_n logsumexp_k( -0.5 * sum_d prec_d (x[n,d]-mu[k,d])^2 )

Math:
  m[n,k]   = cross[n,k] - 0.5*x_sq[n] - 0.5*mu_sq[k],  cross = x @ (prec*mu)^T
  loss     = 0.5*S_xx - sum_n log(rowsum[n]) + N*CC
  rowsum_n = sum_k w''_k * e[n,k],   e[n,k] = exp(cross[n,k] + bias_k)
  bias_k   = C_k - mu_sq[k]/2 with per-k clamp window C_k = mu_sq[k]/2 + 87 - 7.5*sigma_k
  w''_k    = exp(CC - C_k), CC = min_k C_k   (folded into the PE sum weights)

Device mapping (k-major; data-parallel over N, 8 cores):
  - cross^T via PE matmuls: stationary = (prec*mu)^T k-half [128d x 128k],
    moving = x^T slice -> psum [128k, 512n] per (group, k-half); k on the
    psum partition axis makes per-k biases per-partition (free on every
    engine) and lets the PE do the k-reduction
  - exp: ScalarE native Exp with per-partition bias for k-half 0; DVE int16
    Schraudolph exp (one scalar_tensor_tensor, fp32->int16 convert whose
    bits are the bf16 of w*exp) for k-half 1
  - weighted k-sums on PE: stationary = w'' embedded on diag column p of a
    [128, 16] slice, moving = e tiles; all 16 pairs accumulate into one
    [16, 1024] psum region (partition = pair, cols = row-in-pair)
  - Ln(2^-45 * rs) + free-axis accumulate on ScalarE (the ACT Ln table
    degrades above ~2^50, hence the scale); host sums 8 small outputs, adds
    0.5*S_xx (host fp64) and the N*(CC + 45*ln2) shift.
"""

import sys

sys.path.insert(0, "/opt/trn_rl_repo")

from contextlib import ExitStack

import numpy as np

import concourse.bass as bass
import concourse.tile as tile
from concourse import mybir
from concourse.bass_utils import run_bass_kernel_spmd


N, K, D = 131072, 256, 128
NCORES = 8
RPC = N // NCORES  # rows per core = 16384
CHUNK = 2048  # x^T DMA chunk (columns = rows of x)
NCHUNK = RPC // CHUNK  # 8
GROUP = 512  # rows per group (one psum bank per k-half)
NGROUP = RPC // GROUP  # 32
NPAIR = NGROUP // 2  # 16 pairs; pair p -> rowsum partition p

A16 = 128.0 / np.log(2.0)  # Schraudolph slope for bf16-bit target
B16 = 128.0 * 127.0 - 7.3  # mean-error-centered offset
MARGIN = 87.0  # top exp-overflow guard (fp32/bf16 exponent window)
NSIG = 7.5  # sigma margin for the per-k clamp window

F32 = mybir.dt.float32
BF16 = mybir.dt.bfloat16
I16 = mybir.dt.int16
ALU = mybir.AluOpType
ACTF = mybir.ActivationFunctionType


def _split_excess_waits(nc, max_waits=1):
    """Walrus rejects >max_waits sem-waits on one instruction; move the excess
    onto helper Drain instructions on the same engine."""
    import bass_rust

    n_fix = 0
    for f in nc.m.functions:
        for bb in f.blocks:
            insts = bb.instructions
            out_list = []
            changed = False
            for ins in insts:
                si = ins.sync_info
                if si is not None and len(si.on_wait) > max_waits:
                    waits = list(si.on_wait)
                    extra, keep = waits[:-max_waits], waits[-max_waits:]
                    for i in range(0, len(extra), max_waits):
                        nd = mybir.InstDrain(name=f"I-waitfix-{n_fix}", ins=[], outs=[])
                        n_fix += 1
                        nd.engine = ins.engine
                        nd.sync_info = bass_rust.SyncInfo(
                            on_wait=extra[i : i + max_waits], on_update=[]
                        )
                        out_list.append(nd)
                    si.on_wait = keep
                    changed = True
                out_list.append(ins)
            if changed:
                bb.instructions = out_list
    return n_fix


def build_program(apply_waitfix=True, dbg=False):
    nc = bass.Bass("TRN2", target_bir_lowering=False, debug=False)

    xt = nc.dram_tensor("xt", [D, RPC], BF16, kind="ExternalInput").ap()
    mupt = nc.dram_tensor("mupt", [D, K], BF16, kind="ExternalInput").ap()
    # W diag buffers: w''_khalf at column 31, zeros elsewhere; slice
    # [31-s : 63-s] puts w'' on diag column s.
    w0b = nc.dram_tensor("w0b", [D, 64], BF16, kind="ExternalInput").ap()
    w1b = nc.dram_tensor("w1b", [D, 64], BF16, kind="ExternalInput").ap()
    # ScalarE per-partition exp biases: col h = bias vector of k-half h.
    biases = nc.dram_tensor("biases", [D, 2], F32, kind="ExternalInput").ap()
    # DVE stt in1: per-partition A16*bias1_k + B16, broadcast along 512 cols.
    b16 = nc.dram_tensor("b16", [D, 512], F32, kind="ExternalInput").ap()
    out = nc.dram_tensor("out", [32, 2], F32, kind="ExternalOutput").ap()
    if dbg:
        dbg_ps = nc.dram_tensor("dbg_ps", [128, 1024], F32, kind="ExternalOutput").ap()
        dbg_e = nc.dram_tensor("dbg_e", [128, 2048], BF16, kind="ExternalOutput").ap()
        dbg_rs = nc.dram_tensor("dbg_rs", [16, 1024], F32, kind="ExternalOutput").ap()
        dbg_ln = nc.dram_tensor("dbg_ln", [16, 1024], F32, kind="ExternalOutput").ap()

    with tile.TileContext(nc) as tc:
        with ExitStack() as ctx:
            const_pool = ctx.enter_context(tc.tile_pool(name="const", bufs=1))
            xt_pool = ctx.enter_context(tc.tile_pool(name="xtp", bufs=1))
            ps_pool = ctx.enter_context(tc.tile_pool(name="ps", bufs=3, space="PSUM"))
            rs_pool = ctx.enter_context(tc.tile_pool(name="rs", bufs=1, space="PSUM"))
            e_pool = ctx.enter_context(tc.tile_pool(name="e", bufs=6))
            misc_pool = ctx.enter_context(tc.tile_pool(name="misc", bufs=1))

            # DMA: mupt + a small group-0 x tile lead on the SP queue so the
            # PE starts ~4us earlier; chunk 0 follows (group 0's slice in it
            # is simply unused). Scalar HWDGE queue: exp/sum constants first
            # (they gate ScalarE/DVE), then its share of chunks.
            mupt_sb = const_pool.tile([D, K], BF16, tag="mupt")
            nc.sync.dma_start(mupt_sb[:], mupt)
            xtf_sb = xt_pool.tile([D, 2 * GROUP], BF16, tag="xtf")
            nc.sync.dma_start(xtf_sb[:], xt[:, 0 : 2 * GROUP])
            xt_sb = []
            for c in range(NCHUNK):
                xtile = xt_pool.tile([D, CHUNK], BF16, tag=f"xt{c}", name=f"xt{c}")
                xt_sb.append(xtile)
            nc.sync.dma_start(xt_sb[0][:], xt[:, 0:CHUNK])

            bias_sb = const_pool.tile([D, 2], F32, tag="biases")
            nc.scalar.dma_start(bias_sb[:], biases)
            b16_sb = const_pool.tile([D, 512], F32, tag="b16")
            nc.scalar.dma_start(b16_sb[:], b16)
            w0_sb = const_pool.tile([D, 64], BF16, tag="w0")
            nc.scalar.dma_start(w0_sb[:], w0b)
            w1_sb = const_pool.tile([D, 64], BF16, tag="w1")
            nc.scalar.dma_start(w1_sb[:], w1b)

            for c in (1, 3, 5, 7):
                nc.sync.dma_start(xt_sb[c][:], xt[:, c * CHUNK : (c + 1) * CHUNK])
            # chunks 2/4/6 trigger from inside the loop (ScalarE stream) so
            # their ~650ns trigger cost doesn't delay the first exps

            # rowsum accumulator: pair p -> partition p, cols j -> row 1024p+j
            rs = rs_pool.tile([NPAIR, 1024], F32, tag="rs")

            lse_sb = misc_pool.tile([32, 2], F32, tag="lse")
            nc.vector.memset(lse_sb[:], 0.0)
            ln_junk = misc_pool.tile([NPAIR, 1024], F32, tag="lnj")

            e = None
            for g in range(NGROUP):
                p, gi = divmod(g, 2)
                # per-group psum tile [128k, 512n] x 2 k-halves
                ps = ps_pool.tile([128, 1024], F32, tag="ps")
                if gi == 0:
                    # e layout per pair: [gA_h0 | gB_h0 | gA_h1 | gB_h1]
                    e = e_pool.tile([128, 2048], BF16, tag="e")

                n0 = g * GROUP
                ci, off = divmod(n0, CHUNK)
                rhs = (
                    xtf_sb[:, g * GROUP : (g + 1) * GROUP]
                    if g < 2
                    else xt_sb[ci][:, off : off + GROUP]
                )
                for h in range(2):
                    nc.tensor.matmul(
                        ps[:, h * GROUP : (h + 1) * GROUP],
                        lhsT=mupt_sb[:, h * 128 : (h + 1) * 128],
                        rhs=rhs,
                        start=True,
                        stop=True,
                    )

                # k-half 0: ScalarE native exp with per-partition bias
                nc.scalar.activation(
                    e[:, gi * GROUP : (gi + 1) * GROUP],
                    ps[:, 0:GROUP],
                    ACTF.Exp,
                    bias=bias_sb[:, 0:1],
                )
                # k-half 1: DVE int16 Schraudolph exp (writes bf16 bits)
                nc.vector.scalar_tensor_tensor(
                    out=e[:, 1024 + gi * GROUP : 1024 + (gi + 1) * GROUP].bitcast(I16),
                    in0=ps[:, GROUP : 2 * GROUP],
                    scalar=float(A16),
                    in1=b16_sb[:, 0:GROUP],
                    op0=ALU.mult,
                    op1=ALU.add,
                )
                if g in (3, 7, 11):
                    c = (g + 1) // 2  # 2, 4, 6
                    nc.scalar.dma_start(
                        xt_sb[c][:], xt[:, c * CHUNK : (c + 1) * CHUNK]
                    )

                if gi == 1:
                    # weighted k-sums on PE into rs partition p (512-wide
                    # pieces: one mm output may not span psum banks)
                    for wsb, base in ((w0_sb, 0), (w1_sb, 1024)):
                        for c in (0, 512):
                            nc.tensor.matmul(
                                rs[:, c : c + 512],
                                lhsT=wsb[:, 31 - p : 47 - p],
                                rhs=e[:, base + c : base + c + 512],
                                start=(p == 0 and base == 0),
                                stop=(p == NPAIR - 1 and base == 1024),
                                skip_group_check=True,
                            )

            if dbg:
                dbg_rs_sb = misc_pool.tile([16, 1024], F32, tag="dbgrs")
                nc.scalar.copy(dbg_rs_sb[:], rs[:])
                nc.sync.dma_start(dbg_rs, dbg_rs_sb[:])
            # lse: Ln + free-axis accumulate. The ACT Ln table degrades for
            # huge inputs, so scale into a safe window: Ln(2^-45 * rs)
            # = ln(rs) - 45*ln2 (constant restored host-side).
            nc.scalar.activation(
                ln_junk[:],
                rs[:],
                ACTF.Ln,
                scale=float(2.0**-45),
                accum_out=lse_sb[0:NPAIR, 0:1],
            )
            if dbg:
                nc.sync.dma_start(dbg_ln, ln_junk[:])
            nc.sync.dma_start(out, lse_sb[:])

    if apply_waitfix:
        _split_excess_waits(nc)
    return nc


_HOST_CACHE = {}


def _host_prep(x, mu, prec):
    import ml_dtypes

    x = np.asarray(x, dtype=np.float32)
    mu = np.asarray(mu, dtype=np.float32)
    prec = np.asarray(prec, dtype=np.float32)

    muP = mu * prec[None, :]  # [K, D]
    mu_sq = (mu * mu) @ prec  # [K]
    sig = np.sqrt(np.maximum((muP * muP).sum(axis=1), 1e-12))
    C_k = 0.5 * mu_sq + MARGIN - NSIG * sig
    bias_k = (C_k - 0.5 * mu_sq).astype(np.float32)  # = MARGIN - NSIG*sig
    CC = float(C_k.min())
    wpp = np.exp((CC - C_k).astype(np.float64)).astype(np.float32)

    mupt = np.ascontiguousarray(muP.T).astype(ml_dtypes.bfloat16)  # [D, K]

    w0b = np.zeros((D, 64), dtype=ml_dtypes.bfloat16)
    w1b = np.zeros((D, 64), dtype=ml_dtypes.bfloat16)
    w0b[:, 31] = wpp[:128].astype(ml_dtypes.bfloat16)
    w1b[:, 31] = wpp[128:].astype(ml_dtypes.bfloat16)

    biases = np.zeros((D, 2), dtype=np.float32)
    biases[:, 0] = bias_k[:128]
    biases[:, 1] = bias_k[128:]

    b16col = (A16 * bias_k[128:].astype(np.float64) + B16).astype(np.float32)
    b16 = np.ascontiguousarray(np.broadcast_to(b16col[:, None], (D, 512)))

    # host-side exact S_xx and the constant shift
    x_sq = (x.astype(np.float64) * x.astype(np.float64)) @ prec.astype(np.float64)
    s_xx = float(x_sq.sum())

    consts = {
        "mupt": np.ascontiguousarray(mupt),
        "w0b": w0b,
        "w1b": w1b,
        "biases": biases,
        "b16": b16,
    }
    return consts, s_xx, CC


def make_in_maps(x, mu, prec):
    import ml_dtypes

    consts, s_xx, CC = _host_prep(x, mu, prec)
    _HOST_CACHE["s_xx"] = s_xx
    _HOST_CACHE["CC"] = CC

    x = np.asarray(x, dtype=np.float32)
    in_maps = []
    for c in range(NCORES):
        xt_c = np.ascontiguousarray(x[c * RPC : (c + 1) * RPC, :].T).astype(
            ml_dtypes.bfloat16
        )
        m = {"xt": xt_c}
        m.update(consts)
        in_maps.append(m)
    return in_maps


def combine_outputs(outs, prec=None):
    """outs: list of 8 [32, 2] arrays; cols = accumulated ln(rowsum) for pair
    and single slots. loss = 0.5*S_xx - sum(ln) + N*CC."""
    ln_sum = 0.0
    for o in outs:
        o = np.asarray(o, dtype=np.float64)
        ln_sum += o[0:NPAIR, 0].sum()
    ln_sum += N * 45.0 * np.log(2.0)  # undo the device-side Ln input scaling
    total = 0.5 * _HOST_CACHE["s_xx"] - ln_sum + N * _HOST_CACHE["CC"]
    return np.float32(total)


_CACHED_NC = None


def kernel(x, mu, prec):
    global _CACHED_NC
    if _CACHED_NC is None:
        _CACHED_NC = build_program()
    nc = _CACHED_NC
    in_maps = make_in_maps(x, mu, prec)
    res = run_bass_kernel_spmd(nc, in_maps, core_ids=list(range(NCORES)))
    outs = [res.results[c]["out"] for c in range(NCORES)]
    return combine_outputs(outs, prec)


if __name__ == "__main__":
    import reference

    inputs = {k: np.asarray(v) for k, v in reference.setup_inputs().items()}
    expected = float(reference.reference(**inputs))
    actual = float(kernel(**inputs))
    rel = abs(actual - expected) / max(1.0, abs(expected))
    print(f"expected={expected:.6f} actual={actual:.6f} rel={rel:.3e}")


# revision 77
# speedup vs baseline: 1.0336x; 1.0336x over previous
"""Trainium2 Bass kernel for nn_Mixture_24541443129646.

loss(x, mu, prec) = -sum_n logsumexp_k( -0.5 * sum_d prec_d (x[n,d]-mu[k,d])^2 )

Math:
  m[n,k]   = cross[n,k] - 0.5*x_sq[n] - 0.5*mu_sq[k],  cross = x @ (prec*mu)^T
  loss     = 0.5*S_xx - sum_n log(rowsum[n]) + N*CC
  rowsum_n = sum_k w''_k * e[n,k],   e[n,k] = exp(cross[n,k] + bias_k)
  bias_k   = C_k - mu_sq[k]/2 with per-k clamp window C_k = mu_sq[k]/2 + 87 - 7.5*sigma_k
  w''_k    = exp(CC - C_k), CC = min_k C_k   (folded into the PE sum weights)

Device mapping (k-major; data-parallel over N, 8 cores):
  - cross^T via PE matmuls: stationary = (prec*mu)^T k-half [128d x 128k],
    moving = x^T slice -> psum [128k, 512n] per (group, k-half); k on the
    psum partition axis makes per-k biases per-partition (free on every
    engine) and lets the PE do the k-reduction
  - exp: ScalarE native Exp with per-partition bias for k-half 0; DVE int16
    Schraudolph exp (one scalar_tensor_tensor, fp32->int16 convert whose
    bits are the bf16 of w*exp) for k-half 1
  - weighted k-sums on PE: stationary = w'' embedded on diag column p of a
    [128, 16] slice, moving = e tiles; all 16 pairs accumulate into one
    [16, 1024] psum region (partition = pair, cols = row-in-pair)
  - Ln(2^-45 * rs) + free-axis accumulate on ScalarE (the ACT Ln table
    degrades above ~2^50, hence the scale); host sums 8 small outputs, adds
    0.5*S_xx (host fp64) and the N*(CC + 45*ln2) shift.
"""

import sys

sys.path.insert(0, "/opt/trn_rl_repo")

from contextlib import ExitStack

import numpy as np

import concourse.bass as bass
import concourse.tile as tile
from concourse import mybir
from concourse.bass_utils import run_bass_kernel_spmd


N, K, D = 131072, 256, 128
NCORES = 8
RPC = N // NCORES  # rows per core = 16384
CHUNK = 2048  # x^T DMA chunk (columns = rows of x)
NCHUNK = RPC // CHUNK  # 8
GROUP = 512  # rows per group (one psum bank per k-half)
NGROUP = RPC // GROUP  # 32
NPAIR = NGROUP // 2  # 16 pairs; pair p -> rowsum partition p

A16 = 128.0 / np.log(2.0)  # Schraudolph slope for bf16-bit target
B16 = 128.0 * 127.0 - 7.3  # mean-error-centered offset
MARGIN = 87.0  # top exp-overflow guard (fp32/bf16 exponent window)
NSIG = 7.5  # sigma margin for the per-k clamp window

F32 = mybir.dt.float32
BF16 = mybir.dt.bfloat16
I16 = mybir.dt.int16
ALU = mybir.AluOpType
ACTF = mybir.ActivationFunctionType


def _split_excess_waits(nc, max_waits=1):
    """Walrus rejects >max_waits sem-waits on one instruction; move the excess
    onto helper Drain instructions on the same engine."""
    import bass_rust

    n_fix = 0
    for f in nc.m.functions:
        for bb in f.blocks:
            insts = bb.instructions
            out_list = []
            changed = False
            for ins in insts:
                si = ins.sync_info
                if si is not None and len(si.on_wait) > max_waits:
                    waits = list(si.on_wait)
                    extra, keep = waits[:-max_waits], waits[-max_waits:]
                    for i in range(0, len(extra), max_waits):
                        nd = mybir.InstDrain(name=f"I-waitfix-{n_fix}", ins=[], outs=[])
                        n_fix += 1
                        nd.engine = ins.engine
                        nd.sync_info = bass_rust.SyncInfo(
                            on_wait=extra[i : i + max_waits], on_update=[]
                        )
                        out_list.append(nd)
                    si.on_wait = keep
                    changed = True
                out_list.append(ins)
            if changed:
                bb.instructions = out_list
    return n_fix


def build_program(apply_waitfix=True, dbg=False):
    nc = bass.Bass("TRN2", target_bir_lowering=False, debug=False)

    xt = nc.dram_tensor("xt", [D, RPC], BF16, kind="ExternalInput").ap()
    mupt = nc.dram_tensor("mupt", [D, K], BF16, kind="ExternalInput").ap()
    # W diag buffers: w''_khalf at column 31, zeros elsewhere; slice
    # [31-s : 63-s] puts w'' on diag column s.
    w0b = nc.dram_tensor("w0b", [D, 64], BF16, kind="ExternalInput").ap()
    w1b = nc.dram_tensor("w1b", [D, 64], BF16, kind="ExternalInput").ap()
    # ScalarE per-partition exp biases: col h = bias vector of k-half h.
    biases = nc.dram_tensor("biases", [D, 2], F32, kind="ExternalInput").ap()
    # DVE stt in1: per-partition A16*bias1_k + B16, broadcast along 512 cols.
    b16 = nc.dram_tensor("b16", [D, 512], F32, kind="ExternalInput").ap()
    out = nc.dram_tensor("out", [32, 2], F32, kind="ExternalOutput").ap()
    if dbg:
        dbg_ps = nc.dram_tensor("dbg_ps", [128, 1024], F32, kind="ExternalOutput").ap()
        dbg_e = nc.dram_tensor("dbg_e", [128, 2048], BF16, kind="ExternalOutput").ap()
        dbg_rs = nc.dram_tensor("dbg_rs", [16, 1024], F32, kind="ExternalOutput").ap()
        dbg_ln = nc.dram_tensor("dbg_ln", [16, 1024], F32, kind="ExternalOutput").ap()

    with tile.TileContext(nc) as tc:
        with ExitStack() as ctx:
            const_pool = ctx.enter_context(tc.tile_pool(name="const", bufs=1))
            xt_pool = ctx.enter_context(tc.tile_pool(name="xtp", bufs=1))
            ps_pool = ctx.enter_context(tc.tile_pool(name="ps", bufs=3, space="PSUM"))
            rs_pool = ctx.enter_context(tc.tile_pool(name="rs", bufs=1, space="PSUM"))
            e_pool = ctx.enter_context(tc.tile_pool(name="e", bufs=6))
            misc_pool = ctx.enter_context(tc.tile_pool(name="misc", bufs=1))

            # DMA: mupt + a small group-0 x tile lead on the SP queue so the
            # PE starts ~4us earlier; chunk 0 follows (group 0's slice in it
            # is simply unused). Scalar HWDGE queue: exp/sum constants first
            # (they gate ScalarE/DVE), then its share of chunks.
            mupt_sb = const_pool.tile([D, K], BF16, tag="mupt")
            nc.sync.dma_start(mupt_sb[:], mupt)
            xtf_sb = xt_pool.tile([D, 2 * GROUP], BF16, tag="xtf")
            nc.sync.dma_start(xtf_sb[:], xt[:, 0 : 2 * GROUP])
            xt_sb = []
            for c in range(NCHUNK):
                xtile = xt_pool.tile([D, CHUNK], BF16, tag=f"xt{c}", name=f"xt{c}")
                xt_sb.append(xtile)
            nc.sync.dma_start(xt_sb[0][:], xt[:, 0:CHUNK])

            bias_sb = const_pool.tile([D, 2], F32, tag="biases")
            nc.scalar.dma_start(bias_sb[:], biases)
            b16_sb = const_pool.tile([D, 512], F32, tag="b16")
            nc.scalar.dma_start(b16_sb[:], b16)
            w0_sb = const_pool.tile([D, 64], BF16, tag="w0")
            nc.scalar.dma_start(w0_sb[:], w0b)
            w1_sb = const_pool.tile([D, 64], BF16, tag="w1")
            nc.scalar.dma_start(w1_sb[:], w1b)

            for c in (1, 3, 5, 7):
                nc.sync.dma_start(xt_sb[c][:], xt[:, c * CHUNK : (c + 1) * CHUNK])
            # chunks 2/4/6 trigger from inside the loop (ScalarE stream) so
            # their ~650ns trigger cost doesn't delay the first exps

            # rowsum accumulator: group g -> partition g, cols j -> row 512g+j
            rs = rs_pool.tile([NGROUP, 512], F32, tag="rs")

            lse_sb = misc_pool.tile([32, 2], F32, tag="lse")
            nc.vector.memset(lse_sb[:], 0.0)
            ln_junk = misc_pool.tile([NGROUP, 512], F32, tag="lnj")

            e = None
            for g in range(NGROUP):
                p, gi = divmod(g, 2)
                # per-group psum tile [128k, 512n] x 2 k-halves
                ps = ps_pool.tile([128, 1024], F32, tag="ps")
                if gi == 0:
                    # e layout per pair: [gA_h0 | gB_h0 | gA_h1 | gB_h1]
                    e = e_pool.tile([128, 2048], BF16, tag="e")

                n0 = g * GROUP
                ci, off = divmod(n0, CHUNK)
                rhs = (
                    xtf_sb[:, g * GROUP : (g + 1) * GROUP]
                    if g < 2
                    else xt_sb[ci][:, off : off + GROUP]
                )
                for h in range(2):
                    nc.tensor.matmul(
                        ps[:, h * GROUP : (h + 1) * GROUP],
                        lhsT=mupt_sb[:, h * 128 : (h + 1) * 128],
                        rhs=rhs,
                        start=True,
                        stop=True,
                    )

                # k-half 0: ScalarE native exp with per-partition bias
                nc.scalar.activation(
                    e[:, gi * GROUP : (gi + 1) * GROUP],
                    ps[:, 0:GROUP],
                    ACTF.Exp,
                    bias=bias_sb[:, 0:1],
                )
                # k-half 1: DVE int16 Schraudolph exp (writes bf16 bits)
                nc.vector.scalar_tensor_tensor(
                    out=e[:, 1024 + gi * GROUP : 1024 + (gi + 1) * GROUP].bitcast(I16),
                    in0=ps[:, GROUP : 2 * GROUP],
                    scalar=float(A16),
                    in1=b16_sb[:, 0:GROUP],
                    op0=ALU.mult,
                    op1=ALU.add,
                )
                if g in (3, 7, 11):
                    c = (g + 1) // 2  # 2, 4, 6
                    nc.scalar.dma_start(
                        xt_sb[c][:], xt[:, c * CHUNK : (c + 1) * CHUNK]
                    )

                if gi == 1:
                    # weighted k-sums on PE; piece (base, c) -> group slot
                    # s = 2p + c//512 of the single-bank rowsum region
                    for wsb, base in ((w0_sb, 0), (w1_sb, 1024)):
                        for c in (0, 512):
                            s = 2 * p + c // 512
                            nc.tensor.matmul(
                                rs[:, 0:512],
                                lhsT=wsb[:, 31 - s : 63 - s],
                                rhs=e[:, base + c : base + c + 512],
                                start=(p == 0 and base == 0 and c == 0),
                                stop=(p == NPAIR - 1 and base == 1024 and c == 512),
                                skip_group_check=True,
                            )

            if dbg:
                dbg_rs_sb = misc_pool.tile([16, 1024], F32, tag="dbgrs")
                nc.scalar.copy(dbg_rs_sb[:], rs[:])
                nc.sync.dma_start(dbg_rs, dbg_rs_sb[:])
            # lse: Ln + free-axis accumulate. The ACT Ln table degrades for
            # huge inputs, so scale into a safe window: Ln(2^-45 * rs)
            # = ln(rs) - 45*ln2 (constant restored host-side).
            nc.scalar.activation(
                ln_junk[:],
                rs[:],
                ACTF.Ln,
                scale=float(2.0**-45),
                accum_out=lse_sb[0:NGROUP, 0:1],
            )
            if dbg:
                nc.sync.dma_start(dbg_ln, ln_junk[:])
            nc.sync.dma_start(out, lse_sb[:])

    if apply_waitfix:
        _split_excess_waits(nc)
    return nc


_HOST_CACHE = {}


def _host_prep(x, mu, prec):
    import ml_dtypes

    x = np.asarray(x, dtype=np.float32)
    mu = np.asarray(mu, dtype=np.float32)
    prec = np.asarray(prec, dtype=np.float32)

    muP = mu * prec[None, :]  # [K, D]
    mu_sq = (mu * mu) @ prec  # [K]
    sig = np.sqrt(np.maximum((muP * muP).sum(axis=1), 1e-12))
    C_k = 0.5 * mu_sq + MARGIN - NSIG * sig
    bias_k = (C_k - 0.5 * mu_sq).astype(np.float32)  # = MARGIN - NSIG*sig
    CC = float(C_k.min())
    wpp = np.exp((CC - C_k).astype(np.float64)).astype(np.float32)

    mupt = np.ascontiguousarray(muP.T).astype(ml_dtypes.bfloat16)  # [D, K]

    w0b = np.zeros((D, 64), dtype=ml_dtypes.bfloat16)
    w1b = np.zeros((D, 64), dtype=ml_dtypes.bfloat16)
    w0b[:, 31] = wpp[:128].astype(ml_dtypes.bfloat16)
    w1b[:, 31] = wpp[128:].astype(ml_dtypes.bfloat16)

    biases = np.zeros((D, 2), dtype=np.float32)
    biases[:, 0] = bias_k[:128]
    biases[:, 1] = bias_k[128:]

    b16col = (A16 * bias_k[128:].astype(np.float64) + B16).astype(np.float32)
    b16 = np.ascontiguousarray(np.broadcast_to(b16col[:, None], (D, 512)))

    # host-side exact S_xx and the constant shift
    x_sq = (x.astype(np.float64) * x.astype(np.float64)) @ prec.astype(np.float64)
    s_xx = float(x_sq.sum())

    consts = {
        "mupt": np.ascontiguousarray(mupt),
        "w0b": w0b,
        "w1b": w1b,
        "biases": biases,
        "b16": b16,
    }
    return consts, s_xx, CC


def make_in_maps(x, mu, prec):
    import ml_dtypes

    consts, s_xx, CC = _host_prep(x, mu, prec)
    _HOST_CACHE["s_xx"] = s_xx
    _HOST_CACHE["CC"] = CC

    x = np.asarray(x, dtype=np.float32)
    in_maps = []
    for c in range(NCORES):
        xt_c = np.ascontiguousarray(x[c * RPC : (c + 1) * RPC, :].T).astype(
            ml_dtypes.bfloat16
        )
        m = {"xt": xt_c}
        m.update(consts)
        in_maps.append(m)
    return in_maps


def combine_outputs(outs, prec=None):
    """outs: list of 8 [32, 2] arrays; cols = accumulated ln(rowsum) for pair
    and single slots. loss = 0.5*S_xx - sum(ln) + N*CC."""
    ln_sum = 0.0
    for o in outs:
        o = np.asarray(o, dtype=np.float64)
        ln_sum += o[0:NGROUP, 0].sum()
    ln_sum += N * 45.0 * np.log(2.0)  # undo the device-side Ln input scaling
    total = 0.5 * _HOST_CACHE["s_xx"] - ln_sum + N * _HOST_CACHE["CC"]
    return np.float32(total)


_CACHED_NC = None


def kernel(x, mu, prec):
    global _CACHED_NC
    if _CACHED_NC is None:
        _CACHED_NC = build_program()
    nc = _CACHED_NC
    in_maps = make_in_maps(x, mu, prec)
    res = run_bass_kernel_spmd(nc, in_maps, core_ids=list(range(NCORES)))
    outs = [res.results[c]["out"] for c in range(NCORES)]
    return combine_outputs(outs, prec)


if __name__ == "__main__":
    import reference

    inputs = {k: np.asarray(v) for k, v in reference.setup_inputs().items()}
    expected = float(reference.reference(**inputs))
    actual = float(kernel(**inputs))
    rel = abs(actual - expected) / max(1.0, abs(expected))
    print(f"expected={expected:.6f} actual={actual:.6f} rel={rel:.3e}")
